# revision 36
# baseline (speedup 1.0000x reference)
"""Trainium2 Bass kernel for nn_AttentionLayer_48722109006175.

Math: out[b,i,j,h] = softmax_h( leaky_relu( attn_src[b,i,h] + attn_dst[b,j,h]
                                            + edge_dense[b,i,j,h], 0.2 ) )

The edge term is linear in src, so the per-edge scatter collapses:
  edge_dense[b,i,j,:] = cnt[i,j] * (g[b,i,:] - g[b,j,:])
where g = src @ (W_edge @ a_edge) and cnt[i,j] counts (i,j) edges (shared by
all batches).  Likewise attn_src = src @ (W_lin @ a_src), attn_dst =
src @ (W_lin @ a_dst).  So with A = [As|Ad|Ag] [128,12],
sdg = src_b @ A gives all per-node terms.

Per core (one batch b), per 128-row i-tile x 512-col j-block, heads paired:
  PE:  psa = Q = g_i - g_j        (K=2 rank-2 matmul, fp32r)
  DVE: psa = cnt .* psa           (in-place on PSUM)
  PE:  psa += P = s_i + d_j       (K=2 matmul accumulate, fp32)
  ACT: l = prelu(psa, 0.2); m = exp(l)
  POOL: s = sum_h m ; DVE: r ~= 1/s (custom-DVE approx, ~51 ULP)
  DVE/POOL: out[:, j*4+h] = m_h * r  (strided interleave)
Sharding: data-parallel over batch, one batch per NeuronCore.
DMAs are batched (one cnt load / one out store per i-tile) because each
dma_start costs ~0.65us of sequencer issue time.
"""

import numpy as np

B, N, F_IN, H = 8, 1024, 128, 4
JB = 512          # j-block (one PSUM bank at fp32)
NT = N // 128     # 8 i-tiles
NEG_SLOPE = 0.2


def _build_nc(mn_bufs=3, m_bufs=6, ps_bufs=4, o_bufs=3, cnt_bufs=3):
    import concourse.bass as bass
    import concourse.bacc as bacc
    import concourse.mybir as mybir
    import concourse.tile as tile
    from concourse.masks import make_identity

    f32 = mybir.dt.float32
    f32r = mybir.dt.float32r
    AF = mybir.ActivationFunctionType
    OP = mybir.AluOpType

    nc = bacc.Bacc()
    # Reset DMA queues + clear bass-managed semaphores at kernel entry.
    # (Bass only emits this when target_bir_lowering=True; without it, stale
    # semaphore/DMA state from previously-executed NEFFs on the same core
    # races the first tile loads.)
    from concourse.bass import compact_to_ranges
    for sem_range in compact_to_ranges(
        [s for s in nc._kernel_sem_range if s not in nc.barrier_sems]
    ):
        nc.gpsimd.dma_reset(sem_range)
        nc.gpsimd.sem_clear(sem_range)
    nc._nrt_pseudo_barrier()

    src_d = nc.dram_tensor("src", [N, F_IN], f32, kind="ExternalInput")
    cnt_d = nc.dram_tensor("cnt", [N, N], f32, kind="ExternalInput")
    a_d = nc.dram_tensor("A", [F_IN, 12], f32, kind="ExternalInput")
    out_d = nc.dram_tensor("out", [N, N * H], f32, kind="ExternalOutput")

    with tile.TileContext(nc) as tc:
        with tc.tile_pool(name="stage", bufs=1) as stage:
            # Packed staging for the K=2 rank-2 matmuls, head chunks of N:
            #   lhsAll row0 = [s_0..s_3 | g_0..g_3],  row1 = ones
            #   rhsAll row0 = ones,  row1 = [d_0..d_3 | -g_0..-g_3]
            # lhsP_h = lhsAll[:, h*N:], lhsQ_h = lhsAll[:, (4+h)*N:], etc.
            lhsAll = stage.tile([2, 2 * H * N], f32)
            rhsAll = stage.tile([2, 2 * H * N], f32)

            # ---- prologue: srcT via PE transpose, sdgT = (src @ A)^T ----
            with tc.tile_pool(name="pro", bufs=1) as pro, \
                 tc.tile_pool(name="pps", bufs=2, space="PSUM") as pps:
                ones_t = pro.tile([1, 2 * N], f32)
                nc.vector.memset(ones_t[:, 0:N], 1.0)
                nc.vector.memset(ones_t[:, N:], -1.0)
                oap1 = ones_t[0:1, 0:N]
                ones_b = bass.AP(tensor=oap1.tensor, offset=oap1.offset,
                                 ap=[oap1.ap[0], [0, 2 * H], oap1.ap[1]])
                oapm = ones_t[0:1, N:]
                mones_b = bass.AP(tensor=oapm.tensor, offset=oapm.offset,
                                  ap=[oapm.ap[0], [0, H], oapm.ap[1]])
                # lhsAll row1: +1 for P chunks (0..3), -1 for Q chunks (4..7)
                nc.sync.dma_start(lhsAll[1:2, 0:H * N],
                                  bass.AP(tensor=oap1.tensor, offset=oap1.offset,
                                          ap=[oap1.ap[0], [0, H], oap1.ap[1]]))
                nc.sync.dma_start(lhsAll[1:2, H * N:], mones_b)
                nc.sync.dma_start(rhsAll[0:1, :], ones_b)
                ident = pro.tile([128, 128], f32)
                make_identity(nc, ident)
                a_sb = pro.tile([F_IN, 12], f32)
                nc.gpsimd.dma_start(a_sb, a_d[:, :])
                chunks = pro.tile([128, N], f32)
                nc.sync.dma_start(
                    chunks.rearrange("p (c f) -> p c f", c=NT),
                    src_d[:, :].rearrange("(c p) f -> p c f", p=128))
                srcT = pro.tile([128, N], f32)
                sdgT = pro.tile([12, N], f32)
                cview = chunks.rearrange("p (c f) -> p c f", c=NT)
                for c in range(NT):
                    pt = pps.tile([128, 128], f32, tag="pt")
                    nc.tensor.transpose(pt, cview[:, c, :], ident)
                    nc.scalar.copy(srcT[:, c * 128:(c + 1) * 128], pt)
                for half in range(2):
                    ps = pps.tile([12, 512], f32, tag="sdg")
                    nc.tensor.matmul(ps, a_sb,
                                     srcT[:, half * 512:(half + 1) * 512],
                                     start=True, stop=True)
                    nc.scalar.copy(sdgT[:, half * 512:(half + 1) * 512], ps)
                # 4 batched row DMAs fill all staging data rows
                nc.sync.dma_start(lhsAll[0:1, 0:H * N], sdgT[0:4, :])
                nc.sync.dma_start(lhsAll[0:1, H * N:], sdgT[8:12, :])
                nc.sync.dma_start(rhsAll[1:2, 0:H * N], sdgT[4:8, :])
                nc.sync.dma_start(rhsAll[1:2, H * N:], sdgT[8:12, :])

            def lhsQ(h):
                return lhsAll[:, (H + h) * N:(H + h + 1) * N]

            def lhsP(h):
                return lhsAll[:, h * N:(h + 1) * N]

            def rhsQ(h):
                return rhsAll[:, (H + h) * N:(H + h + 1) * N]

            def rhsP(h):
                return rhsAll[:, h * N:(h + 1) * N]

            # ---- main loop ----
            with tc.tile_pool(name="mn", bufs=mn_bufs) as mn, \
                 tc.tile_pool(name="mtiles", bufs=m_bufs) as mpool, \
                 tc.tile_pool(name="ob", bufs=o_bufs) as obp, \
                 tc.tile_pool(name="cntp", bufs=cnt_bufs) as cntp, \
                 tc.tile_pool(name="ps", bufs=ps_bufs, space="PSUM") as psp:
                # recompute i-tile 0 last: its startup-issued version can race
                # semaphore warm-up; the final DRAM write wins.
                order = list(range(NT)) + [0]
                cnt_pref = {}

                def load_cnt(i):
                    t = cntp.tile([128, N], f32, tag="cnt", name=f"cnt{i}")
                    nc.sync.dma_start(t, cnt_d[order[i] * 128:order[i] * 128 + 128, :])
                    return t

                cnt_pref[0] = load_cnt(0)
                for idx, it in enumerate(order):
                    i0 = it * 128
                    cnt_t = cnt_pref.pop(idx)
                    if idx + 1 < len(order):
                        cnt_pref[idx + 1] = load_cnt(idx + 1)
                    o_t = obp.tile([128, N * H], f32, tag="o")
                    o3 = o_t.rearrange("p (j h) -> p j h", h=H)
                    for jb in range(N // JB):
                        j0 = jb * JB
                        mpair = []
                        for hp in range(H // 2):
                            psa = psp.tile([128, 2 * JB], f32, tag="psa")
                            for k in range(2):
                                h = hp * 2 + k
                                nc.tensor.matmul(
                                    psa[:, k * JB:(k + 1) * JB],
                                    lhsQ(h)[:, i0:i0 + 128].bitcast(f32r),
                                    rhsQ(h)[:, j0:j0 + JB].bitcast(f32r),
                                    start=True, stop=True)
                            cs = cnt_t[:, j0:j0 + JB]
                            cnt_b = bass.AP(
                                tensor=cs.tensor, offset=cs.offset,
                                ap=[cs.ap[0], [0, 2], cs.ap[1]])
                            pv = psa.rearrange("p (k j) -> p k j", k=2)
                            nc.vector.tensor_tensor(pv, cnt_b, pv, op=OP.mult)
                            for k in range(2):
                                h = hp * 2 + k
                                nc.tensor.matmul(
                                    psa[:, k * JB:(k + 1) * JB],
                                    lhsP(h)[:, i0:i0 + 128].bitcast(f32r),
                                    rhsP(h)[:, j0:j0 + JB].bitcast(f32r),
                                    start=False, stop=True,
                                    skip_group_check=True)
                            l_t = mn.tile([128, 2 * JB], f32, tag="l")
                            nc.scalar.activation(l_t, psa, AF.Prelu,
                                                 alpha=NEG_SLOPE)
                            m_t = mpool.tile([128, 2 * JB], f32, tag="m")
                            nc.scalar.activation(m_t, l_t, AF.Exp)
                            mpair.append(m_t)
                        s01 = mn.tile([128, JB], f32, tag="s01")
                        nc.gpsimd.tensor_tensor(s01, mpair[0][:, :JB],
                                                mpair[0][:, JB:], op=OP.add)
                        s23 = mn.tile([128, JB], f32, tag="s23")
                        nc.gpsimd.tensor_tensor(s23, mpair[1][:, :JB],
                                                mpair[1][:, JB:], op=OP.add)
                        s = mn.tile([128, JB], f32, tag="s")
                        nc.vector.tensor_tensor(s, s01, s23, op=OP.add)
                        r = mn.tile([128, JB], f32, tag="r")
                        nc.vector.reciprocal_approx_fast(r, s)
                        rap = r[:, :]
                        r_b = bass.AP(tensor=rap.tensor, offset=rap.offset,
                                      ap=[rap.ap[0], [0, 2], rap.ap[1]])
                        oap = o_t[:, :]
                        for hp in range(H // 2):
                            o_pair = bass.AP(
                                tensor=oap.tensor,
                                offset=oap.offset + (j0 * H + hp * 2),
                                ap=[oap.ap[0], [1, 2], [H, JB]])
                            eng = nc.gpsimd if hp == 1 else nc.vector
                            eng.tensor_tensor(
                                o_pair,
                                mpair[hp].rearrange("p (k j) -> p k j", k=2),
                                r_b, op=OP.mult)
                        nc.sync.dma_start(
                            out_d[i0:i0 + 128, j0 * H:(j0 + JB) * H],
                            o_t[:, j0 * H:(j0 + JB) * H])
    nc.finalize()
    return nc


def kernel(src, edge_index, W_lin, a_src, a_dst, W_edge, a_edge):
    from concourse.bass_utils import run_bass_kernel_spmd

    src = np.ascontiguousarray(np.asarray(src, dtype=np.float32))
    ei = np.asarray(edge_index).astype(np.int64)
    W_lin = np.asarray(W_lin, dtype=np.float32)
    a_src = np.asarray(a_src, dtype=np.float32)
    a_dst = np.asarray(a_dst, dtype=np.float32)
    W_edge = np.asarray(W_edge, dtype=np.float32)
    a_edge = np.asarray(a_edge, dtype=np.float32)

    # fold weights: A = [W_lin@a_src | W_lin@a_dst | W_edge@a_edge]  [128,12]
    A = np.concatenate(
        [W_lin @ a_src, W_lin @ a_dst, W_edge @ a_edge], axis=1
    ).astype(np.float32)
    # edge multiplicity matrix (shared across batches)
    cnt = np.zeros((N, N), np.float32)
    np.add.at(cnt, (ei[0], ei[1]), 1.0)

    nc = _build_nc()
    in_maps = [
        {"src": np.ascontiguousarray(src[b]), "cnt": cnt, "A": A}
        for b in range(B)
    ]
    res = run_bass_kernel_spmd(nc, in_maps, core_ids=list(range(B)))
    out = np.stack(
        [res.results[b]["out"].reshape(N, N, H) for b in range(B)], axis=0
    )
    return out


if __name__ == "__main__":
    rng = np.random.default_rng(0)
    inputs = {
        "src": rng.standard_normal((B, N, F_IN), dtype=np.float32),
        "edge_index": rng.integers(0, N, (2, 32768)).astype(np.int32),
        "W_lin": rng.standard_normal((F_IN, 128), dtype=np.float32) / np.sqrt(F_IN),
        "a_src": rng.standard_normal((128, H), dtype=np.float32) / np.sqrt(128),
        "a_dst": rng.standard_normal((128, H), dtype=np.float32) / np.sqrt(128),
        "W_edge": rng.standard_normal((F_IN, 64), dtype=np.float32) / np.sqrt(F_IN),
        "a_edge": rng.standard_normal((64, H), dtype=np.float32) / np.sqrt(64),
    }
    out = kernel(**inputs)
    print("out", out.shape, out.dtype, out.sum())
